# revision 1
# baseline (speedup 1.0000x reference)
"""GNN message passing (gather + segment-sum) on 8 Trainium2 NeuronCores.

Strategy (node-range sharding per the spec's sharding_hint):
  - Destination nodes are range-sharded across the 8 cores (12500 nodes
    each), so each core owns a disjoint slice of the output and no
    cross-core reduction is needed.
  - The device-side gather uses the batched SWDGE row-gather
    (`nc.gpsimd.dma_gather`, 2048 descriptors per call).  Its indices are
    int16, so x is packed as [25001, 256] (4 node rows per packed row plus
    one zero row) and source nodes are split into 4 "colors" by src % 4;
    color q gathers from column slice q*64:(q+1)*64 with elem_step=256 and
    index src//4 <= 25000.
  - Per core and color, the core's nodes are sorted by color-in-degree and
    grouped into 98 tiles of 128 (one SBUF partition per node).
    Accumulation pass k of a tile gathers the k-th color-q source row of
    each node (dummy index -> zero row past a node's degree) and a vector
    add folds it into that color's persistent [128, 98*64] SBUF
    accumulator.  Passes are batched 16 per gather call and pipelined, so
    the kernel is bound by the random-gather HBM traffic (memory
    roofline).
  - Accumulators stream back to DRAM per color; the host undoes the four
    degree-sort permutations, sums the color partials, and concatenates
    the 8 node-range slices.
"""

import numpy as np
from contextlib import ExitStack

import concourse.bacc as bacc
import concourse.bass as bass
import concourse.tile as tile
import concourse.mybir as mybir
from concourse.bass_utils import run_bass_kernel_spmd

N_NODES = 100000
N_EDGES = 1250000
D = 64
N_CORES = 8
NPC = N_NODES // N_CORES          # 12500 nodes per core
P = 128
TILES = (NPC + P - 1) // P        # 98 node tiles per core
NPC_PAD = TILES * P               # 12544
COLORS = 4
RPACK = N_NODES // COLORS + 1     # 25001 packed rows (last = zeros)
DUMMY = RPACK - 1
S = 8                             # gather slots (passes) per dma_gather call (1024-desc SWDGE ring)

# Set by test.py for profiling; harness path leaves these untouched.
PROFILE = False
TRACE_CORES = None
LAST_EXEC_NS = None
LAST_RESULTS = None

_COMPILE_CACHE = {}


def _preprocess(edge_index, x):
    """Host-side sharding: per-core, per-color padded gather-index tables."""
    dest = np.asarray(edge_index[0]).astype(np.int64)
    src = np.asarray(edge_index[1]).astype(np.int64)
    x = np.ascontiguousarray(np.asarray(x), dtype=np.float32)

    x_pack = np.zeros((RPACK, COLORS * D), np.float32)
    x_pack[:N_NODES // COLORS] = x.reshape(N_NODES // COLORS, COLORS * D)

    core_of = dest // NPC
    # per (core, color): (perm, deg_pad, starts_pad, srcs_sorted)
    pc = [[None] * COLORS for _ in range(N_CORES)]
    K_all = np.zeros((N_CORES, COLORS, TILES), np.int64)
    for c in range(N_CORES):
        m = core_of == c
        d_loc = dest[m] - c * NPC
        s_c = src[m]
        color = s_c % COLORS
        for q in range(COLORS):
            mq = color == q
            d_q = d_loc[mq]
            s_q = (s_c[mq] // COLORS).astype(np.int16)
            deg = np.bincount(d_q, minlength=NPC)
            order = np.argsort(d_q, kind="stable")
            s_sorted = s_q[order]
            starts = np.zeros(NPC, np.int64)
            starts[1:] = np.cumsum(deg)[:-1]
            perm = np.argsort(-deg, kind="stable")
            deg_pad = np.concatenate([deg[perm],
                                      np.zeros(NPC_PAD - NPC, np.int64)])
            starts_pad = np.concatenate([starts[perm],
                                         np.zeros(NPC_PAD - NPC, np.int64)])
            K_all[c, q] = deg_pad.reshape(TILES, P)[:, 0]
            pc[c][q] = (perm, deg_pad, starts_pad, s_sorted)

    K = K_all.max(axis=0)                      # [COLORS, TILES] shared schedule

    # color-major slot sequence; each color padded to a multiple of S
    slots = []                                 # (q, t, k) with k=-1 for pad
    calls = []                                 # (q, slot_lo) per call
    for q in range(COLORS):
        q_slots = [(q, t, k) for t in range(TILES) for k in range(K[q][t])]
        while len(q_slots) % S:
            q_slots.append((q, -1, -1))
        for i in range(0, len(q_slots), S):
            calls.append((q, len(slots) + i))
        slots.extend(q_slots)
    n_calls = len(calls)

    idx_maps = []
    for c in range(N_CORES):
        vals = np.full((len(slots), P), DUMMY, np.int16)
        for q in range(COLORS):
            perm, deg_pad, starts_pad, s_sorted = pc[c][q]
            s_safe = np.concatenate([s_sorted, np.zeros(1, np.int16)])
            base = next(i for i, (qq, lo) in enumerate(calls) if qq == q)
            s0 = calls[base][1]
            for t in range(TILES):
                kt = int(K[q][t])
                if kt == 0:
                    continue
                dg = deg_pad[t * P:(t + 1) * P][:, None]
                st = starts_pad[t * P:(t + 1) * P][:, None]
                kk = np.arange(kt)[None, :]
                pos = np.minimum(st + kk, len(s_safe) - 1)
                vals[s0:s0 + kt] = np.where(kk < dg, s_safe[pos], DUMMY).T
                s0 += kt
        # wrap: call block [S, P] -> ravel i=s*128+p -> [16, S*8] -> tile x8
        v = vals.reshape(n_calls, S * P)
        wrapped = v.reshape(n_calls, S * P // 16, 16).transpose(0, 2, 1)
        idx_maps.append(np.ascontiguousarray(
            np.tile(wrapped, (1, 8, 1))))      # [n_calls, 128, S*8]

    perms = [[pc[c][q][0] for q in range(COLORS)] for c in range(N_CORES)]
    K_key = tuple(tuple(int(v) for v in K[q]) for q in range(COLORS))
    return x_pack, idx_maps, perms, K_key, slots, calls


def _build_program(K, slots, calls):
    n_calls = len(calls)
    cols = S * P // 16
    nc = bacc.Bacc("TRN2", target_bir_lowering=False, debug=False,
                   num_devices=N_CORES, num_swdge_queues=4)
    x_dram = nc.dram_tensor("x", [RPACK, COLORS * D], mybir.dt.float32,
                            kind="ExternalInput")
    idx_dram = nc.dram_tensor("idx", [n_calls, P, cols], mybir.dt.int16,
                              kind="ExternalInput")
    out_dram = nc.dram_tensor("out", [COLORS, NPC_PAD, D], mybir.dt.float32,
                              kind="ExternalOutput")

    with tile.TileContext(nc) as tc, ExitStack() as ctx:
        idx_pool = ctx.enter_context(tc.tile_pool(name="idx", bufs=12))
        g_pool = ctx.enter_context(tc.tile_pool(name="g", bufs=12))
        acc_pool = ctx.enter_context(tc.tile_pool(name="acc", bufs=1))

        acc = [acc_pool.tile([P, TILES * D], mybir.dt.float32,
                             tag=f"acc{q}", name=f"acc{q}")
               for q in range(COLORS)]
        for ci, (q, lo) in enumerate(calls):
            idx_t = idx_pool.tile([P, cols], mybir.dt.int16, tag="idx",
                                  name=f"idx{ci}")
            nc.sync.dma_start(out=idx_t[:], in_=idx_dram.ap()[ci])
            g = g_pool.tile([P, S, D], mybir.dt.float32, tag="g",
                            name=f"g{ci}")
            nc.gpsimd.dma_gather(
                out_ap=g[:],
                in_ap=x_dram.ap()[:, q * D:(q + 1) * D],
                idxs_ap=idx_t[:],
                num_idxs=S * P,
                num_idxs_reg=S * P,
                elem_size=D,
                elem_step=COLORS * D,
                queue_num=ci % 4,
            )
            g2 = g[:].rearrange("p s d -> p (s d)")
            for j in range(S):
                qq, t, k = slots[lo + j]
                if t < 0:
                    continue
                src_ap = g2[:, bass.ts(j, D)]
                dst_ap = acc[q][:, bass.ts(t, D)]
                if k == 0:
                    nc.vector.tensor_copy(dst_ap, src_ap)
                else:
                    nc.vector.tensor_add(dst_ap, dst_ap, src_ap)
            # end of this color: zero never-written tiles, then store
            if ci + 1 == n_calls or calls[ci + 1][0] != q:
                for t in range(TILES):
                    if K[q][t] == 0:
                        nc.vector.memset(acc[q][:, bass.ts(t, D)], 0.0)
                nc.sync.dma_start(
                    out=out_dram.ap()[q].rearrange("(t p) d -> p t d", p=P),
                    in_=acc[q][:].rearrange("p (t d) -> p t d", d=D))
    nc.compile()
    return nc


def _install_profile_shim():
    """trace=True under axon needs the NTFF hook that this image's antenv
    lacks; register the ctypes-based one from trn_agent_boot."""
    import sys, types
    import concourse.bass_utils as bu
    if "antenv.axon_hooks" not in sys.modules:
        from trn_agent_boot.trn_boot import _ntff_profile_via_ctypes
        shim = types.ModuleType("antenv.axon_hooks")
        hook = _ntff_profile_via_ctypes("/opt/axon/libaxon_pjrt.so")
        shim.get_axon_ntff_profile_hook = lambda: hook
        shim.set_axon_ntff_profile_hook = lambda h: None
        sys.modules["antenv.axon_hooks"] = shim
    bu.upload_artifacts = lambda tmpdir: f"local:{tmpdir}"


def kernel(edge_index, x):
    global LAST_EXEC_NS, LAST_RESULTS
    x_pack, idx_maps, perms, K, slots, calls = _preprocess(edge_index, x)

    cache_key = (K, len(calls))
    if cache_key not in _COMPILE_CACHE:
        _COMPILE_CACHE[cache_key] = _build_program(K, slots, calls)
    nc = _COMPILE_CACHE[cache_key]

    in_maps = [{"x": x_pack, "idx": idx_maps[c]} for c in range(N_CORES)]
    kwargs = {}
    if PROFILE:
        _install_profile_shim()
        kwargs = dict(trace=True, trace_cores=TRACE_CORES)
    res = run_bass_kernel_spmd(nc, in_maps, core_ids=list(range(N_CORES)),
                               **kwargs)
    LAST_EXEC_NS = res.exec_time_ns
    LAST_RESULTS = res

    out = np.empty((N_NODES, D), np.float32)
    for c in range(N_CORES):
        dev = res.results[c]["out"]            # [COLORS, NPC_PAD, D]
        sl = np.zeros((NPC, D), np.float32)
        for q in range(COLORS):
            tmp = np.empty((NPC, D), np.float32)
            tmp[perms[c][q]] = dev[q][:NPC]
            sl += tmp
        out[c * NPC:(c + 1) * NPC] = sl
    return out

